# revision 16
# baseline (speedup 1.0000x reference)
"""Trainium2 Bass kernel for ChemicalNet (per-species MLP / MoE routing).

Strategy
--------
Only atoms whose species is in {1, 6, 7, 8} produce output (others are 0),
and each such atom only needs ITS OWN species' 3-layer MLP.  The reference
runs all 4 expert networks on all atoms; we route on the host instead:

- host: map species -> expert index, collect per-expert atom index lists
- shard: 2 cores per expert, each core gets half of that expert's atoms
  (the per-core in_map carries that expert's weights, so the single SPMD
  program is expert-agnostic)
- host passes the gathered embedding columns TRANSPOSED ([128, n]) so the
  device needs no transposes: PE contracts over the partition axis directly
- device: L1 matmul+SiLU, L2 matmul (2-step K accum)+SiLU, L3 matmul -> [1,n]
- host scatters the compact per-core outputs back to the full [N, 1] output

Matmul operands travel as bf16 (full TensorEngine rate at any chunk size,
~0.4% per-element precision -- far inside the 2e-2 gate; measured end-to-end
error ~1e-3).  `use_bf16=False` falls back to float32r, `use_f32r=False`
to full fp32.

All input DMAs ride the two HWDGE queues (sync + scalar): the weights are
packed host-side into ONE [128, 770] tensor = one DMA, the embedding
streams in 6 chunks alternating between the queues.  No gpsimd/SWDGE DMAs
anywhere -- SWDGE queues cost ~1us fixed in the TileContext teardown drain.

Per-chunk (512 atoms) the two 128-row halves of the hidden layer land in one
[128, 1024] PSUM tile so a single ACTIVATE applies SiLU to both (the scalar
engine does not pipeline ACTIVATEs; fewer/bigger is faster).  That merge
needs a bias constant along the free axis; biases in this problem are
identically zero, which the host verifies -- nonzero-bias inputs take a
(slower) per-half ACTIVATE path with per-partition bias.

The layer-3 [1, F] matmul accumulates into a corner of the layer-2 PSUM
tile after its ACTIVATE has read it (WAR handled by Tile), so all 8 PSUM
banks go to the 4-deep [128, 1024] pipeline pool.  L3 results collect in
one [1, npad] SBUF tile, written out by a single DMA at the end.

All shapes are compile-time constants derived from the actual input
(the Bass program is built fresh per call).
"""

import numpy as np

import concourse.bass as bass
import concourse.tile as tile
from concourse import bacc, mybir
from concourse.bass_utils import run_bass_kernel_spmd
from concourse.vector_clock import ScopedClock


class FastTileContext(tile.TileContext):
    """TileContext with a minimal end-of-program epilogue.

    The stock ``_drain_and_barrier`` spends ~8.5us resetting every
    semaphore and DMA queue (EVENT_SEMAPHORE_RANGE_CLEAR + quiesce +
    two all-engine barriers) so the NEFF can be re-executed.  This
    kernel's NEFF runs exactly once per compile, so completion safety
    only needs the final drain (which waits for every engine's and
    DMA queue's terminal semaphore value) plus one barrier.
    """

    def _drain_and_barrier(self, tick_clock, wait_clock):
        # No end barrier either: each engine then enters the runtime's
        # fixed ~6us end-of-stream semaphore scan as soon as its own
        # stream ends, so the scans stagger/overlap instead of all
        # starting after the slowest engine.  The sync drain still waits
        # for every terminal semaphore (all DMAs + engines done) before
        # the sync engine ends, which is what completion needs.
        drain_inst = self.nc.sync.drain()
        wait_clock.add_sem_waits(
            drain_inst.ins, ScopedClock({None: tick_clock.global_clock})
        )
        popped = self.nc._tile_sem_poison_stack.pop()
        assert popped is self._sem_poison

N_CORES = 8
NSPECIES = 4
SPECIES_Z = np.array([1, 6, 7, 8], dtype=np.int32)
MAXIDX = 118
D = 128          # embedding dim
H = 256          # hidden dim
F = 512          # atom-chunk size (one PSUM bank of fp32)
FP = mybir.dt.float32
SILU = mybir.ActivationFunctionType.Silu


def _chunk_sizes(npad: int):
    """Ramped chunks: small first chunks so compute starts while the
    embedding stream is still landing (the DMA bus delivers ~80 GB/s
    effective, much slower than compute consumes full chunks), and small
    LAST chunks so little work remains after the final chunk arrives."""
    sizes = []
    for s in (128, 256):
        if sum(sizes) + s <= npad:
            sizes.append(s)
    tail = [128] if npad - sum(sizes) > 256 else []
    body = npad - sum(sizes) - sum(tail)
    while body > F:
        sizes.append(F)
        body -= F
    if body:
        sizes.append(body)
    sizes += tail
    chunks = []
    c0 = 0
    for s in sizes:
        chunks.append((c0, s))
        c0 += s
    return chunks


def _build_program(npad: int, zero_bias: bool, mmdt):
    """One SPMD program: a 3-layer per-expert MLP over `npad` atom columns."""
    nc = bacc.Bacc("TRN2", target_bir_lowering=False, debug=False,
                   num_devices=N_CORES)

    chunks_pre = _chunk_sizes(npad)
    # one contiguous DRAM tensor per chunk: every DMA then reads one
    # fully-contiguous block (the single [D, npad] layout makes each
    # DMA gather 128 strided rows, which the HBM serves noticeably
    # slower than the contiguous case)
    emb_ds = [nc.dram_tensor(f"embT{ci}", [D, f], mmdt, kind="ExternalInput")
              for ci, (c0, f) in enumerate(chunks_pre)]
    # packed weights: [w1 (256) | w2_k0 (256) | w2_k1 (256) | w3 (2)]
    w_d = nc.dram_tensor("w", [D, 770], mmdt, kind="ExternalInput")
    if not zero_bias:
        b1_d = nc.dram_tensor("b1", [128, 2], FP, kind="ExternalInput")
        b2_d = nc.dram_tensor("b2", [128, 2], FP, kind="ExternalInput")
        b3_d = nc.dram_tensor("b3", [1, 1], FP, kind="ExternalInput")
    out_d = nc.dram_tensor("out", [1, npad], FP, kind="ExternalOutput")

    chunks = chunks_pre
    nch = len(chunks)

    with FastTileContext(nc) as tc:
        with (
            tc.tile_pool(name="singles", bufs=1) as singles,
            tc.tile_pool(name="emb", bufs=nch) as embp,
            tc.tile_pool(name="z1p", bufs=nch) as z1p,
            tc.tile_pool(name="z2p", bufs=nch) as z2p,
            tc.tile_pool(name="ps", bufs=3, space="PSUM") as psp,
            tc.tile_pool(name="ps3", bufs=2, space="PSUM") as ps3p,
        ):
            # Preload the SiLU table set while input DMAs run.  The input
            # is a prelude-initialized const AP so this activation has no
            # in-context dependencies: the scheduler keeps it FIRST on the
            # ACT queue, and the table-load pass then emits exactly one
            # LoadActFuncSet (when a DMA leads this queue instead, a
            # second, spurious 1.3us set-0 load appears).
            warm_out = singles.tile([128, 1], mmdt)
            nc.scalar.activation(warm_out[:], nc.const_aps.aps[(FP, 0.0)],
                                 SILU)

            emb_ts = [embp.tile([D, F], mmdt, tag="emb", name=f"emb{ci}")
                      for ci in range(nch)]
            w_t = singles.tile([D, 770], mmdt)

            # Spread input DMAs over three queues so transfers overlap:
            #   sync:   emb0 emb1 emb4 emb6 (+out)
            #   scalar: w1   w2w3 emb3
            #   gpsimd: emb2 emb5
            # (w2|w3 right after w1: MM2(chunk0) needs it early)
            def dma_emb(q, ci):
                c0, f = chunks[ci]
                q.dma_start(emb_ts[ci][:, :f], emb_ds[ci][:])

            nc.scalar.dma_start(w_t[:, :256], w_d[:, :256])
            dma_emb(nc.sync, 0)
            if nch > 1:
                dma_emb(nc.sync, 1)
            nc.scalar.dma_start(w_t[:, 256:], w_d[:, 256:])
            queue_of = {2: nc.gpsimd, 3: nc.scalar, 4: nc.sync,
                        5: nc.gpsimd}
            for ci in range(2, nch):
                dma_emb(queue_of.get(ci, nc.sync), ci)
            if not zero_bias:
                b1_t = singles.tile([128, 2], FP)
                nc.scalar.dma_start(b1_t[:], b1_d[:])
                b2_t = singles.tile([128, 2], FP)
                nc.scalar.dma_start(b2_t[:], b2_d[:])
                b3_t = singles.tile([1, 1], FP)
                nc.scalar.dma_start(b3_t[:], b3_d[:])

            # stationary views into the packed weights tile
            def w1_s(m):
                return w_t[:, m * 128:(m + 1) * 128]

            def w2_s(r, m):
                return w_t[:, 256 + r * 256 + m * 128:256 + r * 256 + (m + 1) * 128]

            w3_s = w_t[:, 768:770]

            out_t = singles.tile([1, npad], FP)

            def m_off(f):
                # matmul output must stay inside one 512-col PSUM bank:
                # pack the m1 half right after m0 only when both fit bank 0
                return f if 2 * f <= F else F

            def act_pair(z_t, ps_t, f, b_t):
                """SiLU both m-halves of a psum tile -> z SBUF.

                Zero-bias path: ONE ACTIVATE over [0, m_off+f) -- for
                off==F > f this also covers the unused gap columns, which
                is harmless and cheaper than a second ACTIVATE."""
                off = m_off(f)
                if zero_bias:
                    nc.scalar.activation(z_t[:, :off + f], ps_t[:, :off + f],
                                         SILU)
                else:
                    for m in range(2):
                        nc.scalar.activation(
                            z_t[:, m * off:m * off + f],
                            ps_t[:, m * off:m * off + f], SILU,
                            bias=b_t[:, m:m + 1])

            # Software-pipelined emission: L1 runs two chunks ahead of L2,
            # L3 one behind, so the scalar engine's in-order ACTIVATE queue
            # never head-of-line blocks (ACT1(c+2) sits between ACT2(c) and
            # ACT2(c+1)) and the PE always has independent matmuls queued.
            z1s, z2s, ps2s = {}, {}, {}

            def emit_l1(ci):
                c0, f = chunks[ci]
                ps1 = psp.tile([128, 2 * F], FP, tag="ps", name=f"ps1_{ci}")
                off = m_off(f)
                for m in range(2):
                    nc.tensor.matmul(ps1[:, m * off:m * off + f],
                                     w1_s(m),
                                     emb_ts[ci][:, :f], start=True, stop=True)
                z1 = z1p.tile([128, 2 * F], mmdt, tag="z1", name=f"z1_{ci}")
                act_pair(z1, ps1, f, None if zero_bias else b1_t)
                z1s[ci] = z1

            def emit_l2(ci):
                c0, f = chunks[ci]
                z1 = z1s[ci]
                off = m_off(f)
                ps2 = psp.tile([128, 2 * F], FP, tag="ps", name=f"ps2_{ci}")
                for m in range(2):
                    nc.tensor.matmul(ps2[:, m * off:m * off + f],
                                     w2_s(0, m),
                                     z1[:, :f], start=True, stop=False)
                    nc.tensor.matmul(ps2[:, m * off:m * off + f],
                                     w2_s(1, m),
                                     z1[:, off:off + f], start=False, stop=True)
                z2 = z2p.tile([128, 2 * F], mmdt, tag="z2", name=f"z2_{ci}")
                act_pair(z2, ps2, f, None if zero_bias else b2_t)
                z2s[ci], ps2s[ci] = z2, ps2

            def emit_l3(ci):
                c0, f = chunks[ci]
                z2 = z2s[ci]
                # L3 gets its own 1-bank PSUM tile so ps2 frees right after
                # its ACTIVATE read -- the 3-deep ps pool never blocks the
                # PE on the (slow, single-partition) DVE output copy.
                ps3 = ps3p.tile([1, F], FP, tag="ps3", name=f"ps3_{ci}")
                off = m_off(f)
                nc.tensor.matmul(ps3[:, :f], w3_s[:, 0:1], z2[:, :f],
                                 start=True, stop=False)
                nc.tensor.matmul(ps3[:, :f], w3_s[:, 1:2], z2[:, off:off + f],
                                 start=False, stop=True)
                if zero_bias:
                    nc.vector.tensor_copy(out_t[:, c0:c0 + f], ps3[:, :f])
                else:
                    nc.vector.tensor_scalar_add(out_t[:, c0:c0 + f],
                                                ps3[:, :f], b3_t[0:1, 0:1])

            # L1 leads L2 by `depth` chunks, EXCEPT the last two chunks'
            # L1s, which are deferred to the final slots: the late chunks
            # arrive off the DMA stream after the ACT engine could have
            # retired earlier chunks' L2s, and the in-order ACTIVATE queue
            # would otherwise head-of-line block several L2 activations
            # behind the final L1s' data waits.
            depth = min(3, nch)
            slot = {k: k - depth for k in range(depth, nch)}
            if nch - 1 >= depth:
                slot[nch - 1] = nch - 2
            if nch - 2 >= depth:
                slot[nch - 2] = nch - 3
            by_slot = {}
            for k, s in slot.items():
                by_slot.setdefault(s, []).append(k)
            for ci in range(min(depth, nch)):
                emit_l1(ci)
            for ci in range(nch):
                emit_l2(ci)
                for k in sorted(by_slot.get(ci, ())):
                    emit_l1(k)
                if ci >= 1:
                    emit_l3(ci - 1)
            emit_l3(nch - 1)

            nc.sync.dma_start(out_d[:], out_t[:])

    nc.compile()
    return nc


def _route(species: np.ndarray):
    """species values -> expert idx (-1 unknown); per-core row assignments."""
    conv = np.full(MAXIDX + 2, -1, dtype=np.int32)
    conv[SPECIES_Z] = np.arange(NSPECIES, dtype=np.int32)
    idx = conv[species]
    core_rows = []
    for s in range(NSPECIES):
        rows = np.flatnonzero(idx == s)
        h = (len(rows) + 1) // 2
        core_rows.append(rows[:h])
        core_rows.append(rows[h:])
    return core_rows


def _run(inputs: dict, trace: bool = False, use_f32r: bool = True,
         use_bf16: bool = True):
    species = inputs["species"]
    embedding = np.ascontiguousarray(inputs["embedding"], dtype=np.float32)
    n_atoms = species.shape[0]
    out_full = np.zeros((n_atoms, 1), dtype=np.float32)

    core_rows = _route(np.asarray(species))
    nmax = max(len(r) for r in core_rows)
    if nmax == 0:
        return out_full, None
    npad = -(-nmax // 16) * 16

    zero_bias = all(
        not np.any(np.asarray(inputs[k])) for k in ("b1", "b2", "b3"))
    if use_bf16:
        mmdt = mybir.dt.bfloat16
    elif use_f32r:
        mmdt = mybir.dt.float32r
    else:
        mmdt = FP
    np_mm = mybir.dt.np(mmdt)
    nc = _build_program(npad, zero_bias, mmdt)

    chunks = _chunk_sizes(npad)
    in_maps = []
    for c in range(N_CORES):
        s = c // 2
        rows = core_rows[c]
        embT = np.zeros((D, npad), dtype=np_mm)
        if len(rows):
            embT[:, :len(rows)] = embedding[rows].T.astype(np_mm)
        w1 = np.asarray(inputs["W1"][s], dtype=np.float32)          # [128, 256]
        w2 = np.asarray(inputs["W2"][s], dtype=np.float32)          # [256, 256]
        w3 = np.asarray(inputs["W3"][s], dtype=np.float32)          # [256, 1]
        wpack = np.concatenate(
            [w1, w2[:128], w2[128:], w3.reshape(2, 128).T], axis=1)
        im = {
            "w": np.ascontiguousarray(wpack.astype(np_mm)),
        }
        for ci, (c0, f) in enumerate(chunks):
            im[f"embT{ci}"] = np.ascontiguousarray(embT[:, c0:c0 + f])
        if not zero_bias:
            im["b1"] = np.ascontiguousarray(
                np.asarray(inputs["b1"][s], dtype=np.float32).reshape(2, 128).T)
            im["b2"] = np.ascontiguousarray(
                np.asarray(inputs["b2"][s], dtype=np.float32).reshape(2, 128).T)
            im["b3"] = np.asarray(inputs["b3"][s], dtype=np.float32).reshape(1, 1)
        in_maps.append(im)

    res = run_bass_kernel_spmd(nc, in_maps, core_ids=list(range(N_CORES)),
                               trace=trace)
    for c in range(N_CORES):
        rows = core_rows[c]
        if len(rows):
            out_full[rows, 0] = res.results[c]["out"][0, :len(rows)]
    return out_full, res


def kernel(**inputs) -> np.ndarray:
    out, _ = _run(inputs, trace=False)
    return out
